# revision 1
# baseline (speedup 1.0000x reference)
"""PiLoraLayer TRN2 kernel: y = x + (alpha/r) * sin((2/pi) * (x @ A) @ B).

x: [4, 4096, 4096] f32; A = A_int8 * scale_A (per-col), B = B_int8 * scale_B
(per-col); rank 16 bottleneck.

Strategy (data-parallel over 8 NeuronCores):
- Host: dequantize the tiny weights once. Fold scale_A, scale_B and 1/pi^2
  into Bp = scale_A[:,None] * B_q * scale_B[None,:] / pi^2; keep A_q as f32.
  Then u = (x @ A_q) @ Bp equals arg/(2*pi) where arg = (2/pi)*h2, and
  y = x + 2*sin(2*pi*u).
- Shard x's 16384 token rows into 8 x [2048, 4096] shards, one per core.
- Device (per core), per 512-token super-tile:
    - DMA x in (4 chunks of [128, 4096]).
    - PE-transpose x into [128h, 512t] slabs; ACT copies PSUM->SBUF.
    - mm1: h1T[16, 512] = sum_k A_k.T @ xT_k (PSUM accumulate, 32 chunks)
    - mm2: per 128-token chunk, u_psum[128, 1024] = h1_c @ Bp_n (2-bank tile)
    - Range reduction (HW Sin LUT only accepts [-pi, pi]):
      k = (u + 1.5*2^23) - 1.5*2^23 in ONE two-op DVE tensor_scalar (RNE
      round-to-integer, written as bf16 which is exact for |k| <= 256);
      PE accumulates -k into the u bank via a bf16 negative-identity matmul,
      leaving frac in [-0.5, 0.5]; ACT computes s = sin(2*pi*frac) -> bf16.
    - DVE: s *= 2 (bf16 4x mode, in place), x_sb += s (mixed f32+bf16),
      DMA x_sb out as y.
- GPSIMD is kept out of the steady-state loop entirely: it is ~10x slower
  than DVE for elementwise work and its SBUF port sharing starves DVE.
"""

import sys

sys.path.insert(0, "/opt/trn_rl_repo")

import numpy as np

import concourse.bacc as bacc
import concourse.bass as bass
import concourse.tile as tile
from concourse import mybir
from concourse.bass import ts
from concourse.bass_utils import run_bass_kernel_spmd

P = 128
HIDDEN = 4096
RANK = 16
N_CORES = 8
TOTAL_ROWS = 4 * 4096
ROWS = TOTAL_ROWS // N_CORES  # 2048 per core
SUPER = 512  # tokens per steady-state super-tile
NCH = SUPER // P  # token chunks per super-tile
KC = HIDDEN // P  # 32 hidden chunks
UBLK = 1024  # tail block width (2 PSUM banks)
NUB = HIDDEN // UBLK  # 4 tail blocks per token chunk
ALPHA_OVER_R = 2.0  # 32.0 / 16
MAGIC = 12582912.0  # 1.5 * 2^23: f32 add/sub rounds to nearest integer
SCALE_2PI = 6.283185  # slightly < 2*pi so the LUT arg stays inside [-pi, pi]

F32 = mybir.dt.float32
F32R = mybir.dt.float32r  # replicated fp32: 1 cycle/row on PE when N>=256
BF16 = mybir.dt.bfloat16


def build_nc(rows: int = ROWS):
    """Build the per-core Bass program for a [rows, 4096] token shard."""
    assert rows % SUPER == 0
    n_super = rows // SUPER

    nc = bacc.Bacc(
        "TRN2",
        target_bir_lowering=False,
        debug=False,
        enable_asserts=False,
        num_devices=N_CORES,
    )
    x_d = nc.dram_tensor("x", [rows, HIDDEN], F32, kind="ExternalInput").ap()
    a_d = nc.dram_tensor("A", [HIDDEN, RANK], F32, kind="ExternalInput").ap()
    bp_d = nc.dram_tensor("Bp", [RANK, HIDDEN], F32, kind="ExternalInput").ap()
    i_d = nc.dram_tensor("I", [P, P], F32, kind="ExternalInput").ap()
    y_d = nc.dram_tensor("out", [rows, HIDDEN], F32, kind="ExternalOutput").ap()

    with tile.TileContext(nc) as tc:
        with (
            tc.tile_pool(name="singles", bufs=1) as singles,
            tc.tile_pool(name="xp", bufs=2) as xpool,
            tc.tile_pool(name="xtp", bufs=6) as xtpool,
            tc.tile_pool(name="kp", bufs=4) as kpool,
            tc.tile_pool(name="sp", bufs=4) as spool,
            tc.tile_pool(name="h1sb", bufs=2) as h1pool,
            tc.tile_pool(name="ptp", bufs=2, space="PSUM") as pt_psum,
            tc.tile_pool(name="h1p", bufs=2, space="PSUM") as h1_psum,
            tc.tile_pool(name="up", bufs=2, space="PSUM") as u_psum,
        ):
            ident = singles.tile([P, P], F32R)
            nc.sync.dma_start(out=ident[:], in_=i_d[:, :].bitcast(F32R))
            nident_bf = singles.tile([P, P], BF16)
            nc.gpsimd.memset(nident_bf[:], 0.0)
            nc.gpsimd.affine_select(
                out=nident_bf[:],
                in_=nident_bf[:],
                compare_op=mybir.AluOpType.not_equal,
                fill=-1.0,
                base=0,
                pattern=[[-1, P]],
                channel_multiplier=1,
            )
            a_sb = singles.tile([P, KC, RANK], F32R)
            nc.sync.dma_start(
                out=a_sb[:],
                in_=a_d.rearrange("(k p) r -> p k r", p=P).bitcast(F32R),
            )
            bp_sb = singles.tile([RANK, HIDDEN], F32R)
            nc.sync.dma_start(out=bp_sb[:], in_=bp_d[:, :].bitcast(F32R))

            def emit_tail_block(state, j):
                """One 1024-wide tail block j for a finished super-tile."""
                x_sb, h1_sb, row0, _nch = state
                c, nb = j // NUB, j % NUB
                u_ps = u_psum.tile([P, UBLK], F32)
                for jj in range(2):
                    nc.tensor.matmul(
                        u_ps[:, ts(jj, 512)],
                        h1_sb[:, ts(c, P)],
                        bp_sb[:, nb * UBLK + jj * 512 : nb * UBLK + (jj + 1) * 512],
                        start=True,
                        stop=True,
                    )
                kq = kpool.tile([P, UBLK], BF16)
                nc.vector.tensor_scalar(
                    kq[:],
                    u_ps[:],
                    MAGIC,
                    MAGIC,
                    mybir.AluOpType.add,
                    mybir.AluOpType.subtract,
                )
                for jj in range(2):
                    nc.tensor.matmul(
                        u_ps[:, ts(jj, 512)],
                        nident_bf[:],
                        kq[:, ts(jj, 512)],
                        start=False,
                        stop=True,
                        skip_group_check=True,
                    )
                s = spool.tile([P, UBLK], BF16)
                nc.scalar.activation(
                    out=s[:],
                    in_=u_ps[:],
                    func=mybir.ActivationFunctionType.Sin,
                    scale=SCALE_2PI,
                )
                nc.vector.tensor_scalar_mul(s[:], s[:], ALPHA_OVER_R)
                nc.vector.tensor_tensor(
                    x_sb[:, c, nb * UBLK : (nb + 1) * UBLK].bitcast(F32R),
                    x_sb[:, c, nb * UBLK : (nb + 1) * UBLK],
                    s[:],
                    mybir.AluOpType.add,
                )
                if nb == NUB - 1:
                    r0 = row0 + c * P
                    nc.gpsimd.dma_start(out=y_d[r0 : r0 + P, :], in_=x_sb[:, c, :])

            # super-tile layout: small first/last tiles halve pipeline
            # fill (k-loop with no tail to hide) and drain (tail with no
            # k-loop to hide)
            layout = []
            r = 0
            sizes = [256] + [SUPER] * ((rows - 512) // SUPER) + [256]
            if rows <= 512:
                sizes = [rows]
            for tok in sizes:
                layout.append((r, tok))
                r += tok
            assert r == rows

            prev = None  # (x_sb, h1_sb, row0, nch) of the previous super-tile

            for st, (row0, tok) in enumerate(layout):
                nch = tok // P
                x_sb = xpool.tile([P, nch, HIDDEN], F32)
                # column-half loads (kb-major) so the first transposes can
                # start after ~1/2 of the super-tile's data has landed
                for kb in range(2):
                    cols = slice(kb * (HIDDEN // 2), (kb + 1) * (HIDDEN // 2))
                    for c in range(nch):
                        r0 = row0 + c * P
                        nc.sync.dma_start(
                            out=x_sb[:, c, cols].bitcast(F32R),
                            in_=x_d[r0 : r0 + P, cols].bitcast(F32R),
                        )

                # mm1 k-loop of st, interleaved with the tail blocks of st-1
                ntail_prev = prev[3] * NUB if prev is not None else 0
                stride = KC // ntail_prev if ntail_prev else 0
                h1_ps = h1_psum.tile([RANK, tok], F32)
                for k in range(KC):
                    pt = pt_psum.tile([P, nch, P], F32R)
                    for c in range(nch):
                        nc.tensor.transpose(
                            pt[:, c, :],
                            x_sb[:, c, ts(k, P)].bitcast(F32R),
                            ident[:],
                        )
                    xt = xtpool.tile([P, tok], F32R)
                    nc.scalar.copy(out=xt[:], in_=pt[:])
                    nc.tensor.matmul(
                        h1_ps[:],
                        a_sb[:, k, :],
                        xt[:],
                        start=(k == 0),
                        stop=(k == KC - 1),
                    )
                    if ntail_prev and k % stride == stride - 1:
                        emit_tail_block(prev, k // stride)
                h1_sb = h1pool.tile([RANK, tok], F32R)
                nc.vector.tensor_copy(h1_sb[:], h1_ps[:])
                prev = (x_sb, h1_sb, row0, nch)

            # drain: the last super-tile's tail has no successor to hide in
            for j in range(prev[3] * NUB):
                emit_tail_block(prev, j)

    nc.compile()
    return nc


_NC_CACHE: dict[int, object] = {}


def _get_nc(rows: int = ROWS):
    nc = _NC_CACHE.get(rows)
    if nc is None:
        nc = build_nc(rows)
        _NC_CACHE[rows] = nc
    return nc


def _prep_weights(A_int8, B_int8, scale_A, scale_B):
    a_f = np.ascontiguousarray(A_int8.astype(np.float32))
    bp = np.ascontiguousarray(
        scale_A.astype(np.float32)[:, None]
        * B_int8.astype(np.float32)
        * scale_B.astype(np.float32)[None, :]
        * np.float32(1.0 / (np.pi * np.pi))
    )
    return a_f, bp


def kernel(x, A_int8, B_int8, scale_A, scale_B):
    x = np.asarray(x)
    orig_shape = x.shape
    xf = np.ascontiguousarray(x.reshape(TOTAL_ROWS, HIDDEN).astype(np.float32))
    a_f, bp = _prep_weights(
        np.asarray(A_int8), np.asarray(B_int8), np.asarray(scale_A), np.asarray(scale_B)
    )

    nc = _get_nc(ROWS)
    eye = np.eye(P, dtype=np.float32)
    in_maps = [
        {"x": xf[i * ROWS : (i + 1) * ROWS], "A": a_f, "Bp": bp, "I": eye}
        for i in range(N_CORES)
    ]
    res = run_bass_kernel_spmd(nc, in_maps, core_ids=list(range(N_CORES)))
    y = np.concatenate([r["out"] for r in res.results], axis=0)
    return y.reshape(orig_shape).astype(np.float32)



# revision 6
# speedup vs baseline: 1.5073x; 1.5073x over previous
"""PiLoraLayer TRN2 kernel: y = x + (alpha/r) * sin((2/pi) * (x @ A) @ B).

x: [4, 4096, 4096] f32; A = A_int8 * scale_A (per-col), B = B_int8 * scale_B
(per-col); rank 16 bottleneck.  alpha/r = 2.

v2 strategy (data-parallel over 8 NeuronCores, transposed fp16 streaming):
- Host: shard x's 16384 token rows into 8 shards; stage xh = (x/2) as fp16,
  TRANSPOSED to [4096, 2048] per core.  This kills the on-device PE
  transpose pass and its PSUM->SBUF copy pass of v1 (~210us PE + ~100us
  ACT), and fp16 halves DMA traffic both ways.  fp16 x costs ~3e-3 rel err
  (budget 2e-2); bf16 would cost ~5e-2.
- Weights: A_q as exact fp16 ints; Bp = 2 * scale_A[:,None] * B_q *
  scale_B[None,:] / pi^2 (f32), so u := (xh @ A) @ Bp = arg/(2*pi) with
  arg = (2/pi)*h the true sin argument.
- Device per core (xT fully SBUF-resident, 128 KB/partition):
  - mm1: h1[16, tok] = sum_k A_k.T @ xh[k-chunk, tok]  (fp16, PSUM acc)
  - mm2 (transposed out): u[128, tok] per H-chunk = Bp_c.T @ h1  (f32r)
  - range reduction (Sin LUT domain is [-pi, pi]): k = (u+1.5*2^23)-1.5*2^23
    in one DVE tensor_scalar (PSUM-read, bf16 out, exact for |k|<=256);
    PE accumulates -k via bf16 negative-identity matmul -> frac in [-.5,.5]
  - ACT: s = sin(2*pi*frac) -> fp16, 1024-wide calls (2 H-chunks paired)
  - residual: yh = xh + s in place (plain fp16 tensor_tensor add; the *2 is
    folded out host-side).  Adds alternate DVE/GPSIMD to balance: DVE also
    carries the krounds (PSUM-read, 1x mode, ~78us), GPSIMD does nothing
    else but is ~4x slower per element, so a 50/50 pair split lands both
    near ~95-100us.
  - DMA yh out as fp16 (sync/HWDGE).
- Host: y = 2 * float32(yh).T  (exact exponent shift).
"""

import sys

sys.path.insert(0, "/opt/trn_rl_repo")

import numpy as np

import concourse.bacc as bacc
import concourse.tile as tile
from concourse import mybir
from concourse.bass_utils import run_bass_kernel_spmd

P = 128
HIDDEN = 4096
RANK = 16
KC = HIDDEN // P  # 32 hidden chunks
N_CORES = 8
TOTAL_ROWS = 4 * 4096
ROWS = TOTAL_ROWS // N_CORES  # 2048 tokens per core
TB = 512  # steady-state token block
NPAIR = KC // 2  # 16 sin/kround groups of 2 H-chunks
MAGIC = 12582912.0  # 1.5 * 2^23: f32 add/sub rounds to nearest integer
SCALE_2PI = 6.283185  # slightly < 2*pi so the LUT arg stays inside [-pi, pi]

F32 = mybir.dt.float32
F32R = mybir.dt.float32r
BF16 = mybir.dt.bfloat16
FP16 = mybir.dt.float16

# fraction of residual-add pairs that run on DVE (rest on GPSIMD)
DVE_ADD_NUM = 1
DVE_ADD_DEN = 2


def build_nc(rows: int = ROWS):
    """Per-core Bass program for a transposed [4096, rows] fp16 token shard."""
    nc = bacc.Bacc(
        "TRN2",
        target_bir_lowering=False,
        debug=False,
        enable_asserts=False,
        num_devices=N_CORES,
    )
    x_d = nc.dram_tensor("x", [HIDDEN, rows], FP16, kind="ExternalInput").ap()
    a_d = nc.dram_tensor("A", [HIDDEN, RANK], FP16, kind="ExternalInput").ap()
    bp_d = nc.dram_tensor("Bp", [RANK, HIDDEN], F32, kind="ExternalInput").ap()
    y_d = nc.dram_tensor("out", [HIDDEN, rows], FP16, kind="ExternalOutput").ap()

    x_r = x_d.rearrange("(k p) t -> p k t", p=P)
    y_r = y_d.rearrange("(k p) t -> p k t", p=P)

    with tile.TileContext(nc) as tc:
        with (
            tc.tile_pool(name="singles", bufs=1) as singles,
            tc.tile_pool(name="h1sb", bufs=2) as h1sb_pool,
            tc.tile_pool(name="kqp", bufs=3) as kq_pool,
            tc.tile_pool(name="sp", bufs=3) as s_pool,
            tc.tile_pool(name="h1p", bufs=2, space="PSUM") as h1_psum,
            tc.tile_pool(name="up", bufs=3, space="PSUM") as u_psum,
        ):
            nident_bf = singles.tile([P, P], BF16)
            nc.gpsimd.memset(nident_bf[:], 0.0)
            nc.gpsimd.affine_select(
                out=nident_bf[:],
                in_=nident_bf[:],
                compare_op=mybir.AluOpType.not_equal,
                fill=-1.0,
                base=0,
                pattern=[[-1, P]],
                channel_multiplier=1,
            )
            a_sb = singles.tile([P, KC, RANK], FP16)
            nc.sync.dma_start(
                out=a_sb[:], in_=a_d.rearrange("(k p) r -> p k r", p=P)
            )
            bp_sb = singles.tile([RANK, HIDDEN], F32R)
            nc.sync.dma_start(out=bp_sb[:], in_=bp_d[:, :].bitcast(F32R))

            # resident x (fp16): 128 KB/partition for rows=2048
            xs = singles.tile([P, KC, rows], FP16)

            # token-block layout; small edge blocks halve pipeline fill/drain
            layout = []
            r = 0
            if rows <= TB:
                sizes = [rows]
            else:
                sizes = [TB // 2] + [TB] * ((rows - TB) // TB) + [TB // 2]
            for tok in sizes:
                layout.append((r, tok))
                r += tok
            assert r == rows

            # stage all input DMAs up front; mm1 of block b waits only on its
            # own slice via tile dependency tracking
            for row0, tok in layout:
                nc.sync.dma_start(
                    out=xs[:, :, row0 : row0 + tok],
                    in_=x_r[:, :, row0 : row0 + tok],
                )

            adds = 0

            def finish_pair(st):
                """-k accumulate, sin, residual add for a built pair."""
                nonlocal adds
                u, kq, pair, row0, tok = st
                for j in range(2):
                    nc.tensor.matmul(
                        u[:, j, :tok],
                        nident_bf[:],
                        kq[:, j, :],
                        start=False,
                        stop=True,
                        skip_group_check=True,
                    )
                s = s_pool.tile([P, 2, tok], FP16)
                nc.scalar.activation(
                    out=s[:],
                    in_=u[:, :, :tok],
                    func=mybir.ActivationFunctionType.Sin,
                    scale=SCALE_2PI,
                )
                c0 = pair * 2
                eng = nc.vector if (adds % DVE_ADD_DEN) < DVE_ADD_NUM else nc.gpsimd
                adds += 1
                eng.tensor_tensor(
                    xs[:, c0 : c0 + 2, row0 : row0 + tok],
                    s[:],
                    xs[:, c0 : c0 + 2, row0 : row0 + tok],
                    mybir.AluOpType.add,
                )
                if pair % 2 == 1:
                    d0 = (pair - 1) * 2
                    nc.sync.dma_start(
                        out=y_r[:, d0 : d0 + 4, row0 : row0 + tok],
                        in_=xs[:, d0 : d0 + 4, row0 : row0 + tok],
                    )

            pending = None  # last built-but-unfinished pair

            for row0, tok in layout:
                h1_ps = h1_psum.tile([RANK, tok], F32)
                for k in range(KC):
                    nc.tensor.matmul(
                        h1_ps[:],
                        a_sb[:, k, :],
                        xs[:, k, row0 : row0 + tok],
                        start=(k == 0),
                        stop=(k == KC - 1),
                    )
                h1_sb = h1sb_pool.tile([RANK, tok], F32R)
                nc.vector.tensor_copy(h1_sb[:], h1_ps[:])

                for pair in range(NPAIR):
                    # one PSUM bank (512 f32) per H-chunk: accumulation
                    # groups are bank-granular, so two chunks must never
                    # share a bank (start=True would clear the sibling's
                    # has_written and break the -k accumulate)
                    u = u_psum.tile([P, 2, max(tok, 512)], F32)
                    for j in range(2):
                        c = pair * 2 + j
                        nc.tensor.matmul(
                            u[:, j, :tok],
                            bp_sb[:, c * P : (c + 1) * P],
                            h1_sb[:],
                            start=True,
                            stop=True,
                        )
                    kq = kq_pool.tile([P, 2, tok], BF16)
                    nc.vector.tensor_scalar(
                        kq[:],
                        u[:, :, :tok],
                        MAGIC,
                        MAGIC,
                        mybir.AluOpType.add,
                        mybir.AluOpType.subtract,
                    )
                    if pending is not None:
                        finish_pair(pending)
                    pending = (u, kq, pair, row0, tok)

            finish_pair(pending)

    nc.compile()
    return nc


_NC_CACHE: dict[int, object] = {}


def _get_nc(rows: int = ROWS):
    nc = _NC_CACHE.get(rows)
    if nc is None:
        nc = build_nc(rows)
        _NC_CACHE[rows] = nc
    return nc


def _prep_weights(A_int8, B_int8, scale_A, scale_B):
    a16 = np.ascontiguousarray(A_int8.astype(np.float16))  # ints <=127: exact
    bp = np.ascontiguousarray(
        scale_A.astype(np.float32)[:, None]
        * B_int8.astype(np.float32)
        * scale_B.astype(np.float32)[None, :]
        * np.float32(2.0 / (np.pi * np.pi))  # extra *2 compensates x/2 staging
    )
    return a16, bp


def _shard_inputs(x, A_int8, B_int8, scale_A, scale_B):
    xf = x.reshape(TOTAL_ROWS, HIDDEN)
    xh = (xf.astype(np.float32) * np.float32(0.5)).astype(np.float16)
    a16, bp = _prep_weights(A_int8, B_int8, scale_A, scale_B)
    in_maps = []
    for i in range(N_CORES):
        xt = np.ascontiguousarray(xh[i * ROWS : (i + 1) * ROWS].T)
        in_maps.append({"x": xt, "A": a16, "Bp": bp})
    return in_maps


def _gather_output(res, orig_shape):
    y = np.empty((TOTAL_ROWS, HIDDEN), dtype=np.float32)
    for i in range(N_CORES):
        # device computed yh = x/2 + sin(...); y = 2*yh (exact x2 in f32)
        y[i * ROWS : (i + 1) * ROWS] = res.results[i]["out"].T
    y *= np.float32(2.0)
    return y.reshape(orig_shape)


def kernel(x, A_int8, B_int8, scale_A, scale_B):
    x = np.asarray(x)
    orig_shape = x.shape
    in_maps = _shard_inputs(
        x,
        np.asarray(A_int8),
        np.asarray(B_int8),
        np.asarray(scale_A),
        np.asarray(scale_B),
    )
    nc = _get_nc(ROWS)
    res = run_bass_kernel_spmd(nc, in_maps, core_ids=list(range(N_CORES)))
    return _gather_output(res, orig_shape)


# revision 11
# speedup vs baseline: 1.6960x; 1.1252x over previous
"""PiLoraLayer TRN2 kernel: y = x + (alpha/r) * sin((2/pi) * (x @ A) @ B).

x: [4, 4096, 4096] f32; A = A_int8 * scale_A (per-col), B = B_int8 * scale_B
(per-col); rank 16 bottleneck.  alpha/r = 2.

v2 strategy (data-parallel over 8 NeuronCores, transposed fp16 streaming):
- Host: shard x's 16384 token rows into 8 shards; stage xh = (x/2) as fp16,
  TRANSPOSED to [4096, 2048] per core.  This kills the on-device PE
  transpose pass and its PSUM->SBUF copy pass of v1 (~210us PE + ~100us
  ACT), and fp16 halves DMA traffic both ways.  fp16 x costs ~3e-3 rel err
  (budget 2e-2); bf16 would cost ~5e-2.
- Weights: A_q as exact fp16 ints; Bp = 2 * scale_A[:,None] * B_q *
  scale_B[None,:] / pi^2 (f32), so u := (xh @ A) @ Bp = arg/(2*pi) with
  arg = (2/pi)*h the true sin argument.
- Device per core (xT fully SBUF-resident, 128 KB/partition):
  - mm1: h1[16, tok] = sum_k A_k.T @ xh[k-chunk, tok]  (fp16, PSUM acc)
  - mm2 (transposed out): u[128, tok] per H-chunk = Bp_c.T @ h1  (f32r)
  - range reduction (Sin LUT domain is [-pi, pi]): k = (u+1.5*2^23)-1.5*2^23
    in one DVE tensor_scalar (PSUM-read, bf16 out, exact for |k|<=256);
    PE accumulates -k via bf16 negative-identity matmul -> frac in [-.5,.5]
  - ACT: s = sin(2*pi*frac) -> fp16, 1024-wide calls (2 H-chunks paired)
  - residual: yh = xh + s in place (plain fp16 tensor_tensor add; the *2 is
    folded out host-side).  Adds alternate DVE/GPSIMD to balance: DVE also
    carries the krounds (PSUM-read, 1x mode, ~78us), GPSIMD does nothing
    else but is ~4x slower per element, so a 50/50 pair split lands both
    near ~95-100us.
  - DMA yh out as fp16 (sync/HWDGE).
- Host: y = 2 * float32(yh).T  (exact exponent shift).
"""

import sys

sys.path.insert(0, "/opt/trn_rl_repo")

import numpy as np

import concourse.bacc as bacc
import concourse.tile as tile
from concourse import mybir
from concourse.bass_utils import run_bass_kernel_spmd

P = 128
HIDDEN = 4096
RANK = 16
KC = HIDDEN // P  # 32 hidden chunks
N_CORES = 8
TOTAL_ROWS = 4 * 4096
ROWS = TOTAL_ROWS // N_CORES  # 2048 tokens per core
TB = 512  # steady-state token block
NPAIR = KC // 2  # 16 sin/kround groups of 2 H-chunks
MAGIC = 12582912.0  # 1.5 * 2^23: f32 add/sub rounds to nearest integer
SCALE_2PI = 6.283185  # slightly < 2*pi so the LUT arg stays inside [-pi, pi]

F32 = mybir.dt.float32
F32R = mybir.dt.float32r
BF16 = mybir.dt.bfloat16
FP16 = mybir.dt.float16

# fraction of residual-add pairs that run on DVE (rest on GPSIMD).  DVE also
# owns the kround pass (~81us); GPSIMD only does adds but is ~2.8x slower per
# element, so a ~1/5 : 4/5 split lands both near ~90us.
DVE_ADD_NUM = 1
DVE_ADD_DEN = 5


def build_nc(rows: int = ROWS):
    """Per-core Bass program for a transposed [4096, rows] fp16 token shard."""
    nc = bacc.Bacc(
        "TRN2",
        target_bir_lowering=False,
        debug=False,
        enable_asserts=False,
        num_devices=N_CORES,
    )
    x_d = nc.dram_tensor("x", [HIDDEN, rows], FP16, kind="ExternalInput").ap()
    a_d = nc.dram_tensor("A", [HIDDEN, RANK], FP16, kind="ExternalInput").ap()
    bp_d = nc.dram_tensor("Bp", [RANK, HIDDEN], FP16, kind="ExternalInput").ap()
    y_d = nc.dram_tensor("out", [HIDDEN, rows], FP16, kind="ExternalOutput").ap()

    x_r = x_d.rearrange("(k p) t -> p k t", p=P)
    y_r = y_d.rearrange("(k p) t -> p k t", p=P)

    with tile.TileContext(nc) as tc:
        with (
            tc.tile_pool(name="singles", bufs=1) as singles,
            tc.tile_pool(name="h1sb", bufs=2) as h1sb_pool,
            tc.tile_pool(name="kqp", bufs=3) as kq_pool,
            tc.tile_pool(name="sp", bufs=3) as s_pool,
            tc.tile_pool(name="h1p", bufs=2, space="PSUM") as h1_psum,
            tc.tile_pool(name="up", bufs=3, space="PSUM") as u_psum,
        ):
            nident_bf = singles.tile([P, P], BF16)
            nc.gpsimd.memset(nident_bf[:], 0.0)
            nc.gpsimd.affine_select(
                out=nident_bf[:],
                in_=nident_bf[:],
                compare_op=mybir.AluOpType.not_equal,
                fill=-1.0,
                base=0,
                pattern=[[-1, P]],
                channel_multiplier=1,
            )
            a_sb = singles.tile([P, KC, RANK], FP16)
            nc.sync.dma_start(
                out=a_sb[:], in_=a_d.rearrange("(k p) r -> p k r", p=P)
            )
            bp_sb = singles.tile([RANK, HIDDEN], FP16)
            nc.sync.dma_start(out=bp_sb[:], in_=bp_d[:, :])

            # resident x (fp16): 128 KB/partition for rows=2048
            xs = singles.tile([P, KC, rows], FP16)

            # token-block layout; small edge blocks halve pipeline fill/drain
            layout = []
            r = 0
            if rows <= TB:
                sizes = [rows]
            else:
                sizes = [TB // 2] + [TB] * ((rows - TB) // TB) + [TB // 2]
            for tok in sizes:
                layout.append((r, tok))
                r += tok
            assert r == rows

            # stage all input DMAs up front; mm1 of block b waits only on its
            # own slice via tile dependency tracking
            for row0, tok in layout:
                nc.sync.dma_start(
                    out=xs[:, :, row0 : row0 + tok],
                    in_=x_r[:, :, row0 : row0 + tok],
                )

            adds = 0

            def finish_pair(st):
                """-k accumulate, sin, residual add for a built pair."""
                nonlocal adds
                u, kq, pair, row0, tok = st
                for j in range(2):
                    nc.tensor.matmul(
                        u[:, j, :tok],
                        nident_bf[:],
                        kq[:, j, :],
                        start=False,
                        stop=True,
                        skip_group_check=True,
                    )
                s = s_pool.tile([P, 2, tok], FP16)
                nc.scalar.activation(
                    out=s[:],
                    in_=u[:, :, :tok],
                    func=mybir.ActivationFunctionType.Sin,
                    scale=SCALE_2PI,
                )
                c0 = pair * 2
                eng = nc.vector if (adds % DVE_ADD_DEN) < DVE_ADD_NUM else nc.gpsimd
                adds += 1
                eng.tensor_tensor(
                    xs[:, c0 : c0 + 2, row0 : row0 + tok],
                    s[:],
                    xs[:, c0 : c0 + 2, row0 : row0 + tok],
                    mybir.AluOpType.add,
                )
                if pair % 2 == 1:
                    d0 = (pair - 1) * 2
                    nc.sync.dma_start(
                        out=y_r[:, d0 : d0 + 4, row0 : row0 + tok],
                        in_=xs[:, d0 : d0 + 4, row0 : row0 + tok],
                    )

            pending = None  # last built-but-unfinished pair

            for row0, tok in layout:
                h1_ps = h1_psum.tile([RANK, tok], F32)
                for k in range(KC):
                    nc.tensor.matmul(
                        h1_ps[:],
                        a_sb[:, k, :],
                        xs[:, k, row0 : row0 + tok],
                        start=(k == 0),
                        stop=(k == KC - 1),
                    )
                h1_sb = h1sb_pool.tile([RANK, tok], FP16)
                nc.vector.tensor_copy(h1_sb[:], h1_ps[:])

                for pair in range(NPAIR):
                    # one PSUM bank (512 f32) per H-chunk: accumulation
                    # groups are bank-granular, so two chunks must never
                    # share a bank (start=True would clear the sibling's
                    # has_written and break the -k accumulate)
                    u = u_psum.tile([P, 2, max(tok, 512)], F32)
                    for j in range(2):
                        c = pair * 2 + j
                        nc.tensor.matmul(
                            u[:, j, :tok],
                            bp_sb[:, c * P : (c + 1) * P],
                            h1_sb[:],
                            start=True,
                            stop=True,
                        )
                    kq = kq_pool.tile([P, 2, tok], BF16)
                    nc.vector.tensor_scalar(
                        kq[:],
                        u[:, :, :tok],
                        MAGIC,
                        MAGIC,
                        mybir.AluOpType.add,
                        mybir.AluOpType.subtract,
                    )
                    if pending is not None:
                        finish_pair(pending)
                    pending = (u, kq, pair, row0, tok)

            finish_pair(pending)

    nc.compile()
    return nc


_NC_CACHE: dict[int, object] = {}


def _get_nc(rows: int = ROWS):
    nc = _NC_CACHE.get(rows)
    if nc is None:
        nc = build_nc(rows)
        _NC_CACHE[rows] = nc
    return nc


def _prep_weights(A_int8, B_int8, scale_A, scale_B):
    # A/1024 is exact in fp16 (|A|<=127 ints, exponent shift); the
    # compensating *1024 on Bp lifts its tiny entries into fp16's normal
    # range (min |Bp| ~2e-4 > 6.1e-5).  fp16 mm2 streams 1 cyc/col on PE
    # where f32r ran 4-pass.
    a16 = np.ascontiguousarray(
        (A_int8.astype(np.float32) * np.float32(1.0 / 1024.0)).astype(np.float16)
    )
    bp = np.ascontiguousarray(
        (
            scale_A.astype(np.float32)[:, None]
            * B_int8.astype(np.float32)
            * scale_B.astype(np.float32)[None, :]
            # 2.0: compensates x/2 staging; 1024: compensates A/1024
            * np.float32(2048.0 / (np.pi * np.pi))
        ).astype(np.float16)
    )
    return a16, bp


def _shard_inputs(x, A_int8, B_int8, scale_A, scale_B):
    xf = x.reshape(TOTAL_ROWS, HIDDEN)
    xh = (xf.astype(np.float32) * np.float32(0.5)).astype(np.float16)
    a16, bp = _prep_weights(A_int8, B_int8, scale_A, scale_B)
    in_maps = []
    for i in range(N_CORES):
        xt = np.ascontiguousarray(xh[i * ROWS : (i + 1) * ROWS].T)
        in_maps.append({"x": xt, "A": a16, "Bp": bp})
    return in_maps


def _gather_output(res, orig_shape):
    y = np.empty((TOTAL_ROWS, HIDDEN), dtype=np.float32)
    for i in range(N_CORES):
        # device computed yh = x/2 + sin(...); y = 2*yh (exact x2 in f32)
        y[i * ROWS : (i + 1) * ROWS] = res.results[i]["out"].T
    y *= np.float32(2.0)
    return y.reshape(orig_shape)


def kernel(x, A_int8, B_int8, scale_A, scale_B):
    x = np.asarray(x)
    orig_shape = x.shape
    in_maps = _shard_inputs(
        x,
        np.asarray(A_int8),
        np.asarray(B_int8),
        np.asarray(scale_A),
        np.asarray(scale_B),
    )
    nc = _get_nc(ROWS)
    res = run_bass_kernel_spmd(nc, in_maps, core_ids=list(range(N_CORES)))
    return _gather_output(res, orig_shape)
